# Initial kernel scaffold
#
"""ContextNorm Trainium2 kernel.

Per-context BatchNorm (training-mode batch stats, biased variance) with an
affine transform and a 1/sqrt(prior) scale, distributed data-parallel over
the sample dim across 8 NeuronCores with a device-side AllReduce of the
per-context segment sums (s1, s2).

Device program (SPMD, identical on all 8 cores; per-core inputs differ):
  phase 1: stream local x chunks [128, 512]; accumulate, via one-hot
           matmuls on the PE, s1 = oh^T @ x and s2 = oh^T @ x^2 in PSUM.
  phase 2: AllReduce [16, 1024] (s1|s2) over the 8 cores.
  phase 3: compute w = gamma*rsqrt(var+eps)*rsqrt(prior) and
           b = (beta - gamma*mean*rsqrt(var+eps))*rsqrt(prior) on-chip
           ([16, 512] each), with an absent-context blend so that
           absent contexts give out == x (w=1, b=0).
  phase 4: per chunk, gather w/b rows by context via one-hot matmuls
           (wg = oh @ w, bg = oh @ b in PSUM) and apply
           out = x*wg + bg on the DVE; stream results out.

Host side: shards rows, builds packed one-hot tensors and tiny per-context
constants (1/count, present*rsqrt(prior), 1-present) from `contexts`.
"""

import sys

sys.path.insert(0, "/opt/trn_rl_repo")

import numpy as np

import concourse.bass as bass
import concourse.tile as tile
from concourse import mybir
from concourse.bass_utils import run_bass_kernel_spmd

N, C, K = 131072, 512, 16
NCORES = 8
NLOC = N // NCORES            # 16384 rows per core
P = 128                       # rows per chunk (SBUF partitions)
NCHUNK = NLOC // P            # 128 chunks per core
G = 4                         # chunks per DMA group (1 MiB transfers)
NGROUP = NCHUNK // G          # 32 groups
NCACHE_G = 10                 # groups kept resident in SBUF between passes
EPS = 1e-3
F32 = mybir.dt.float32


def build_nc():
    nc = bass.Bass(num_devices=NCORES)

    x = nc.dram_tensor("x", [NLOC, C], F32, kind="ExternalInput")
    # oh_p[p, c*K + k] = 1.0 if contexts[c*128 + p] == k else 0.0
    oh_p = nc.dram_tensor("oh_p", [P, NCHUNK * K], F32, kind="ExternalInput")
    # ohT_p[k, c*P + p] = same one-hot, transposed layout for the apply pass
    ohT_p = nc.dram_tensor("ohT_p", [K, NCHUNK * P], F32, kind="ExternalInput")
    gamma = nc.dram_tensor("gamma", [K, C], F32, kind="ExternalInput")
    beta = nc.dram_tensor("beta", [K, C], F32, kind="ExternalInput")
    invn = nc.dram_tensor("invn", [K, 1], F32, kind="ExternalInput")
    psp = nc.dram_tensor("psp", [K, 1], F32, kind="ExternalInput")      # present*rsqrt(prior)
    ompres = nc.dram_tensor("ompres", [K, 1], F32, kind="ExternalInput")  # 1-present
    out = nc.dram_tensor("out", [NLOC, C], F32, kind="ExternalOutput")

    # DRAM views grouped for 1 MiB DMAs: row = g*(G*P) + j*P + p
    x_v = x[:, :].rearrange("(g j p) c -> g p j c", p=P, j=G)
    out_v = out[:, :].rearrange("(g j p) c -> g p j c", p=P, j=G)

    with tile.TileContext(nc) as tc:
        with (
            tc.tile_pool(name="consts", bufs=1) as consts,
            tc.tile_pool(name="xcache", bufs=1) as xcache,
            tc.tile_pool(name="xstream", bufs=3) as xstream,
            tc.tile_pool(name="xsq", bufs=3) as xsq_pool,
            tc.tile_pool(name="ohtg", bufs=3) as ohtg_pool,
            tc.tile_pool(name="spsum", bufs=1, space="PSUM") as spsum,
            tc.tile_pool(name="apsum", bufs=2, space="PSUM") as apsum,
            tc.tile_pool(name="dram", bufs=1, space="DRAM") as dram,
        ):
            # ---- constants in ----
            oh_sb = consts.tile([P, NCHUNK * K], F32)
            nc.sync.dma_start(out=oh_sb, in_=oh_p[:, :])
            gamma_sb = consts.tile([K, C], F32)
            nc.sync.dma_start(out=gamma_sb, in_=gamma[:, :])
            beta_sb = consts.tile([K, C], F32)
            nc.sync.dma_start(out=beta_sb, in_=beta[:, :])
            invn_sb = consts.tile([K, 1], F32)
            nc.sync.dma_start(out=invn_sb, in_=invn[:, :])
            psp_sb = consts.tile([K, 1], F32)
            nc.sync.dma_start(out=psp_sb, in_=psp[:, :])
            ompres_sb = consts.tile([K, 1], F32)
            nc.sync.dma_start(out=ompres_sb, in_=ompres[:, :])
            eps_sb = consts.tile([K, 1], F32)
            nc.vector.memset(eps_sb, EPS)

            # ---- phase 1: local segment sums into PSUM ----
            s1 = spsum.tile([K, C], F32)
            s2 = spsum.tile([K, C], F32)
            x_tiles = {}
            for g in range(NGROUP):
                if g < NCACHE_G:
                    xt = xcache.tile([P, G, C], F32, tag=f"xc{g}")
                    x_tiles[g] = xt
                else:
                    xt = xstream.tile([P, G, C], F32, tag="xs")
                nc.sync.dma_start(out=xt, in_=x_v[g])
                for j in range(G):
                    c = g * G + j
                    xsq = xsq_pool.tile([P, C], F32)
                    nc.scalar.activation(
                        out=xsq, in_=xt[:, j, :],
                        func=mybir.ActivationFunctionType.Square,
                    )
                    nc.tensor.matmul(
                        s1, lhsT=oh_sb[:, c * K:(c + 1) * K], rhs=xt[:, j, :],
                        start=(c == 0), stop=(c == NCHUNK - 1),
                    )
                    nc.tensor.matmul(
                        s2, lhsT=oh_sb[:, c * K:(c + 1) * K], rhs=xsq,
                        start=(c == 0), stop=(c == NCHUNK - 1),
                    )

            # ---- phase 2: AllReduce of (s1 | s2) ----
            s_sb = consts.tile([K, 2 * C], F32)
            nc.vector.tensor_copy(out=s_sb[:, 0:C], in_=s1)
            nc.vector.tensor_copy(out=s_sb[:, C:2 * C], in_=s2)
            cc_in = dram.tile([K, 2 * C], F32)
            cc_out = dram.tile([K, 2 * C], F32)
            nc.sync.dma_start(out=cc_in, in_=s_sb)
            nc.gpsimd.collective_compute(
                "AllReduce",
                mybir.AluOpType.add,
                replica_groups=[list(range(NCORES))],
                ins=[cc_in.opt()],
                outs=[cc_out.opt()],
            )
            sg = consts.tile([K, 2 * C], F32)
            nc.sync.dma_start(out=sg, in_=cc_out)
            s1g = sg[:, 0:C]
            s2g = sg[:, C:2 * C]

            # ---- phase 3: per-(context, channel) scale/shift ----
            mean = consts.tile([K, C], F32)
            nc.vector.tensor_scalar_mul(out=mean, in0=s1g, scalar1=invn_sb)
            ex2 = consts.tile([K, C], F32)
            nc.vector.tensor_scalar_mul(out=ex2, in0=s2g, scalar1=invn_sb)
            m2 = consts.tile([K, C], F32)
            nc.vector.tensor_mul(out=m2, in0=mean, in1=mean)
            var = consts.tile([K, C], F32)
            nc.vector.tensor_sub(out=var, in0=ex2, in1=m2)
            std = consts.tile([K, C], F32)
            nc.scalar.activation(
                out=std, in_=var, func=mybir.ActivationFunctionType.Sqrt,
                bias=eps_sb,
            )
            istd = consts.tile([K, C], F32)
            nc.vector.reciprocal(out=istd, in_=std)
            wv = consts.tile([K, C], F32)
            nc.vector.tensor_mul(out=wv, in0=gamma_sb, in1=istd)
            w_sb = consts.tile([K, C], F32)
            nc.vector.tensor_scalar(
                out=w_sb, in0=wv, scalar1=psp_sb, scalar2=ompres_sb,
                op0=mybir.AluOpType.mult, op1=mybir.AluOpType.add,
            )
            mi = consts.tile([K, C], F32)
            nc.vector.tensor_mul(out=mi, in0=mean, in1=istd)
            gmi = consts.tile([K, C], F32)
            nc.vector.tensor_mul(out=gmi, in0=gamma_sb, in1=mi)
            bt = consts.tile([K, C], F32)
            nc.vector.tensor_sub(out=bt, in0=beta_sb, in1=gmi)
            b_sb = consts.tile([K, C], F32)
            nc.vector.tensor_scalar_mul(out=b_sb, in0=bt, scalar1=psp_sb)

            # ---- phase 4: apply out = x * w[ctx] + b[ctx] ----
            ohT_v = ohT_p[:, :].rearrange("k (g q) -> g k q", q=G * P)
            for g in range(NGROUP):
                if g in x_tiles:
                    xt = x_tiles[g]
                else:
                    xt = xstream.tile([P, G, C], F32, tag="xs")
                    nc.sync.dma_start(out=xt, in_=x_v[g])
                ohtg = ohtg_pool.tile([K, G * P], F32)
                nc.sync.dma_start(out=ohtg, in_=ohT_v[g])
                for j in range(G):
                    wg = apsum.tile([P, C], F32, tag="wg")
                    bg = apsum.tile([P, C], F32, tag="bg")
                    nc.tensor.matmul(
                        wg, lhsT=ohtg[:, j * P:(j + 1) * P], rhs=w_sb,
                        start=True, stop=True,
                    )
                    nc.tensor.matmul(
                        bg, lhsT=ohtg[:, j * P:(j + 1) * P], rhs=b_sb,
                        start=True, stop=True,
                    )
                    nc.vector.tensor_mul(out=xt[:, j, :], in0=xt[:, j, :], in1=wg)
                    nc.vector.tensor_add(out=xt[:, j, :], in0=xt[:, j, :], in1=bg)
                nc.sync.dma_start(out=out_v[g], in_=xt)

    return nc


_NC = None


def _get_nc():
    global _NC
    if _NC is None:
        _NC = build_nc()
    return _NC


def _host_prep(samples, contexts, gamma, beta, priors):
    samples = np.ascontiguousarray(np.asarray(samples, dtype=np.float32))
    contexts = np.asarray(contexts, dtype=np.int32)
    gamma = np.ascontiguousarray(np.asarray(gamma, dtype=np.float32))
    beta = np.ascontiguousarray(np.asarray(beta, dtype=np.float32))
    priors = np.asarray(priors, dtype=np.float32)

    counts = np.bincount(contexts, minlength=K).astype(np.float32)[:K]
    present = (counts > 0).astype(np.float32)
    invn = (1.0 / np.maximum(counts, 1.0)).astype(np.float32).reshape(K, 1)
    psp = (present / np.sqrt(priors)).astype(np.float32).reshape(K, 1)
    ompres = (1.0 - present).astype(np.float32).reshape(K, 1)

    in_maps = []
    for r in range(NCORES):
        lo, hi = r * NLOC, (r + 1) * NLOC
        ctx_shard = contexts[lo:hi]
        one = (ctx_shard[:, None] == np.arange(K, dtype=np.int32)).astype(np.float32)
        oh_p = np.ascontiguousarray(
            one.reshape(NCHUNK, P, K).transpose(1, 0, 2).reshape(P, NCHUNK * K)
        )
        ohT_p = np.ascontiguousarray(one.T.reshape(K, NCHUNK * P))
        in_maps.append({
            "x": samples[lo:hi],
            "oh_p": oh_p,
            "ohT_p": ohT_p,
            "gamma": gamma,
            "beta": beta,
            "invn": invn,
            "psp": psp,
            "ompres": ompres,
        })
    return in_maps


def run(samples, contexts, gamma, beta, priors, **run_kwargs):
    """Run the SPMD kernel; returns (full_output, BassKernelResults)."""
    in_maps = _host_prep(samples, contexts, gamma, beta, priors)
    nc = _get_nc()
    res = run_bass_kernel_spmd(nc, in_maps, core_ids=list(range(NCORES)), **run_kwargs)
    out = np.concatenate([res.results[r]["out"] for r in range(NCORES)], axis=0)
    return np.ascontiguousarray(out, dtype=np.float32), res


def kernel(samples, contexts, gamma, beta, priors):
    out, _ = run(samples, contexts, gamma, beta, priors)
    return out


# revision 21
# speedup vs baseline: 1.8542x; 1.8542x over previous
"""ContextNorm Trainium2 kernel.

Per-context BatchNorm (training-mode batch stats, biased variance) with an
affine transform and a 1/sqrt(prior) scale, distributed data-parallel over
the sample dim across 8 NeuronCores with a device-side AllReduce of the
per-context segment sums (s1, s2).

Device program (SPMD, identical on all 8 cores; per-core inputs differ):
  phase 1: stream local x chunks [128, 512] as float32r (SWDGE cast);
           accumulate, via one-hot matmuls on the PE at full rate,
           s1 = oh^T @ x and s2 = oh^T @ x^2 in PSUM.  float32r rounds
           x to a 12-bit mantissa; the rounding noise averages out over
           ~8k samples per context (stats error ~1e-6).
  phase 2: AllReduce [16, 1024] (s1|s2) over the 8 cores.
  phase 3: compute w = gamma*rsqrt(var+eps)*rsqrt(prior) and
           b = (beta - gamma*mean*rsqrt(var+eps))*rsqrt(prior) on-chip
           ([16, 512] each), with an absent-context blend so that absent
           contexts give out == x (w=1, b=0).  Split w into an exact
           float32r (hi, lo) pair via SWDGE cast + subtract.
  phase 4: per chunk, gather w/b rows by context via float32r one-hot
           matmuls (wg = oh@w_hi + oh@w_lo accumulated in PSUM — exact;
           bg = oh@b_r) and apply out = x*wg + bg on the DVE with x
           re-read in full fp32; stream results out.

Host side: shards rows, builds packed one-hot tensors and tiny per-context
constants (1/count, present*rsqrt(prior), 1-present) from `contexts`.
"""

import sys

sys.path.insert(0, "/opt/trn_rl_repo")

import types

import numpy as np
import orjson

import concourse.bass as bass
import concourse.tile as tile
from concourse import mybir
from concourse.bass_utils import run_bass_kernel_spmd

N, C, K = 131072, 512, 16
NCORES = 8
NLOC = N // NCORES            # 16384 rows per core
P = 128                       # rows per chunk (SBUF partitions)
NCHUNK = NLOC // P            # 128 chunks per core
G = 4                         # chunks per DMA group (1 MiB transfers)
NGROUP = NCHUNK // G          # 32 groups
NCACHE_G = 8                  # groups kept resident in SBUF (fp32) between passes
EPS = 1e-3
F32 = mybir.dt.float32
F32R = mybir.dt.float32r
BF16 = mybir.dt.bfloat16

# Walrus codegen in this toolchain encodes at most ONE semaphore wait per
# instruction ("Too many sync wait commands" otherwise), while the Tile
# scheduler freely attaches several.  Split the extras into standalone
# EventSemaphore instructions placed directly before the owner in the same
# engine queue — semantically identical (waits execute in queue order).
_HOIST_SKIP = {"EventSemaphore"}


def _split_multi_waits(d: dict) -> None:
    counter = 0
    for f in d["functions"]:
        for blk in f["blocks"]:
            new = []
            for ins in blk["instructions"]:
                si = ins.get("sync_info") or {}
                waits = si.get("on_wait") or []
                if len(waits) > 1 and ins.get("opcode") not in _HOIST_SKIP:
                    for wentry in waits[:-1]:
                        counter += 1
                        new.append({
                            "debug": ins.get("debug", 0),
                            "engine": ins["engine"],
                            "ins": [],
                            "name": f"hoistw-{counter}",
                            "opcode": "EventSemaphore",
                            "outs": [],
                            "sync_info": {"on_wait": [wentry]},
                        })
                    si["on_wait"] = [waits[-1]]
                new.append(ins)
            blk["instructions"] = new


def _patched_to_json_bytes(self) -> bytes:
    d = orjson.loads(mybir.module_to_json_bytes(self.m))
    _split_multi_waits(d)
    return orjson.dumps(d)


def build_nc():
    nc = bass.Bass(num_devices=NCORES)

    x = nc.dram_tensor("x", [NLOC, C], F32, kind="ExternalInput")
    # oh_p[p, c*K + k] = 1.0 if contexts[c*128 + p] == k else 0.0
    oh_p = nc.dram_tensor("oh_p", [P, NCHUNK * K], F32, kind="ExternalInput")
    # ohT3_p[32*s + k, c*P + p] = one-hot transposed, replicated in strips at
    # partitions 0/32/64 (32-aligned starts; pad rows are zero) for the
    # single-matmul bf16 w gather
    ohT3_p = nc.dram_tensor("ohT3_p", [96, NCHUNK * P], BF16, kind="ExternalInput")
    gamma = nc.dram_tensor("gamma", [K, C], F32, kind="ExternalInput")
    beta = nc.dram_tensor("beta", [K, C], F32, kind="ExternalInput")
    invn = nc.dram_tensor("invn", [K, 1], F32, kind="ExternalInput")
    psp = nc.dram_tensor("psp", [K, 1], F32, kind="ExternalInput")      # present*rsqrt(prior)
    ompres = nc.dram_tensor("ompres", [K, 1], F32, kind="ExternalInput")  # 1-present
    out = nc.dram_tensor("out", [NLOC, C], F32, kind="ExternalOutput")

    # DRAM views grouped for 1 MiB DMAs: row = g*(G*P) + j*P + p
    x_v = x[:, :].rearrange("(g j p) c -> g p j c", p=P, j=G)
    out_v = out[:, :].rearrange("(g j p) c -> g p j c", p=P, j=G)

    with tile.TileContext(nc) as tc:
        with (
            tc.tile_pool(name="consts", bufs=1) as consts,
            tc.tile_pool(name="xcache", bufs=1) as xcache,
            tc.tile_pool(name="xrstream", bufs=3) as xrstream,
            tc.tile_pool(name="xstream", bufs=4) as xstream,
            tc.tile_pool(name="xsq", bufs=3) as xsq_pool,
            tc.tile_pool(name="ohtg", bufs=4) as ohtg_pool,
            tc.tile_pool(name="dram", bufs=1, space="DRAM") as dram,
        ):
            # ---- constants in ----
            oh_sb = consts.tile([P, NCHUNK * K], F32R)
            nc.gpsimd.dma_start(out=oh_sb, in_=oh_p[:, :])
            gamma_sb = consts.tile([K, C], F32)
            nc.sync.dma_start(out=gamma_sb, in_=gamma[:, :])
            beta_sb = consts.tile([K, C], F32)
            nc.sync.dma_start(out=beta_sb, in_=beta[:, :])
            invn_sb = consts.tile([K, 1], F32)
            nc.sync.dma_start(out=invn_sb, in_=invn[:, :])
            psp_sb = consts.tile([K, 1], F32)
            nc.sync.dma_start(out=psp_sb, in_=psp[:, :])
            ompres_sb = consts.tile([K, 1], F32)
            nc.sync.dma_start(out=ompres_sb, in_=ompres[:, :])
            eps_sb = consts.tile([K, 1], F32)
            nc.vector.memset(eps_sb, EPS)

            # ---- phase 1: local segment sums into PSUM (float32r) ----
            # x groups stream in as fp32 at full HWDGE line rate; the DVE
            # (idle in this phase) makes the float32r copies the stats
            # matmuls need.  The first NCACHE_G groups stay resident for
            # phase 4; the rest are re-read then.
            x_tiles = {}
            s_sb = consts.tile([K, 2 * C], F32)
            with tc.tile_pool(name="spsum", bufs=1, space="PSUM") as spsum:
                s1 = spsum.tile([K, C], F32)
                s2 = spsum.tile([K, C], F32)
                for g in range(NGROUP):
                    if g < NCACHE_G:
                        xt = xcache.tile([P, G, C], F32, tag=f"xc{g}")
                    else:
                        xt = xstream.tile([P, G, C], F32, tag="xs")
                    nc.sync.dma_start(out=xt, in_=x_v[g])
                    if g < NCACHE_G:
                        x_tiles[g] = xt
                    xr = xrstream.tile([P, G, C], F32R, tag="xr")
                    nc.vector.tensor_copy(out=xr, in_=xt)
                    for j in range(G):
                        c = g * G + j
                        xsq = xsq_pool.tile([P, C], F32R)
                        nc.scalar.activation(
                            out=xsq, in_=xr[:, j, :],
                            func=mybir.ActivationFunctionType.Square,
                        )
                        nc.tensor.matmul(
                            s1, lhsT=oh_sb[:, c * K:(c + 1) * K], rhs=xr[:, j, :],
                            start=(c == 0), stop=(c == NCHUNK - 1),
                        )
                        nc.tensor.matmul(
                            s2, lhsT=oh_sb[:, c * K:(c + 1) * K], rhs=xsq,
                            start=(c == 0), stop=(c == NCHUNK - 1),
                        )

                # ---- phase 2: AllReduce of (s1 | s2) ----
                nc.vector.tensor_copy(out=s_sb[:, 0:C], in_=s1)
                nc.vector.tensor_copy(out=s_sb[:, C:2 * C], in_=s2)
            cc_in = dram.tile([K, 2 * C], F32)
            cc_out = dram.tile([K, 2 * C], F32)
            nc.sync.dma_start(out=cc_in, in_=s_sb)
            nc.gpsimd.collective_compute(
                "AllReduce",
                mybir.AluOpType.add,
                replica_groups=[list(range(NCORES))],
                ins=[cc_in.opt()],
                outs=[cc_out.opt()],
            )
            sg = consts.tile([K, 2 * C], F32)
            nc.sync.dma_start(out=sg, in_=cc_out)
            s1g = sg[:, 0:C]
            s2g = sg[:, C:2 * C]

            # ---- phase 3: per-(context, channel) scale/shift ----
            mean = consts.tile([K, C], F32)
            nc.vector.tensor_scalar_mul(out=mean, in0=s1g, scalar1=invn_sb)
            ex2 = consts.tile([K, C], F32)
            nc.vector.tensor_scalar_mul(out=ex2, in0=s2g, scalar1=invn_sb)
            m2 = consts.tile([K, C], F32)
            nc.vector.tensor_mul(out=m2, in0=mean, in1=mean)
            var = consts.tile([K, C], F32)
            nc.vector.tensor_sub(out=var, in0=ex2, in1=m2)
            std = consts.tile([K, C], F32)
            nc.scalar.activation(
                out=std, in_=var, func=mybir.ActivationFunctionType.Sqrt,
                bias=eps_sb,
            )
            istd = consts.tile([K, C], F32)
            nc.vector.reciprocal(out=istd, in_=std)
            wv = consts.tile([K, C], F32)
            nc.vector.tensor_mul(out=wv, in0=gamma_sb, in1=istd)
            w_sb = consts.tile([K, C], F32)
            nc.vector.tensor_scalar(
                out=w_sb, in0=wv, scalar1=psp_sb, scalar2=ompres_sb,
                op0=mybir.AluOpType.mult, op1=mybir.AluOpType.add,
            )
            mi = consts.tile([K, C], F32)
            nc.vector.tensor_mul(out=mi, in0=mean, in1=istd)
            gmi = consts.tile([K, C], F32)
            nc.vector.tensor_mul(out=gmi, in0=gamma_sb, in1=mi)
            bt = consts.tile([K, C], F32)
            nc.vector.tensor_sub(out=bt, in0=beta_sb, in1=gmi)
            b_sb = consts.tile([K, C], F32)
            nc.vector.tensor_scalar_mul(out=b_sb, in0=bt, scalar1=psp_sb)

            # Exact bf16 3-way split of w (h+m+l carries ~27 mantissa bits;
            # the one-hot products are exact and PSUM accumulates in fp32)
            # and 2-way split of b, staged at partition 0 via DVE casts and
            # DMA'd into 32-aligned strips for the stacked gather matmul.
            wst = consts.tile([96, C], BF16)
            bst = consts.tile([64, C], BF16)
            nc.vector.memset(wst, 0.0)
            nc.vector.memset(bst, 0.0)
            w_h = consts.tile([K, C], BF16)
            nc.vector.tensor_copy(out=w_h, in_=w_sb)
            wm_f = consts.tile([K, C], F32)
            nc.vector.tensor_sub(out=wm_f, in0=w_sb, in1=w_h)
            w_m = consts.tile([K, C], BF16)
            nc.vector.tensor_copy(out=w_m, in_=wm_f)
            wl_f = consts.tile([K, C], F32)
            nc.vector.tensor_sub(out=wl_f, in0=wm_f, in1=w_m)
            w_l = consts.tile([K, C], BF16)
            nc.vector.tensor_copy(out=w_l, in_=wl_f)
            b_h = consts.tile([K, C], BF16)
            nc.vector.tensor_copy(out=b_h, in_=b_sb)
            bm_f = consts.tile([K, C], F32)
            nc.vector.tensor_sub(out=bm_f, in0=b_sb, in1=b_h)
            b_m = consts.tile([K, C], BF16)
            nc.vector.tensor_copy(out=b_m, in_=bm_f)
            nc.sync.dma_start(out=wst[0:K, :], in_=w_h)
            nc.sync.dma_start(out=wst[32:32 + K, :], in_=w_m)
            nc.sync.dma_start(out=wst[64:64 + K, :], in_=w_l)
            nc.scalar.dma_start(out=bst[0:K, :], in_=b_h)
            nc.scalar.dma_start(out=bst[32:32 + K, :], in_=b_m)

            # ---- phase 4: apply out = x * w[ctx] + b[ctx] ----
            ohT_v = ohT3_p[:, :].rearrange("k (g q) -> g k q", q=G * P)
            with tc.tile_pool(name="apsum", bufs=4, space="PSUM") as apsum:
                for g in range(NGROUP):
                    if g in x_tiles:
                        xt = x_tiles[g]
                    else:
                        xt = xstream.tile([P, G, C], F32, tag="xs")
                        nc.sync.dma_start(out=xt, in_=x_v[g])
                    ohtg = ohtg_pool.tile([96, G * P], BF16)
                    nc.sync.dma_start(out=ohtg, in_=ohT_v[g])
                    for j in range(G):
                        wg = apsum.tile([P, C], F32, tag="wg")
                        bg = apsum.tile([P, C], F32, tag="bg")
                        nc.tensor.matmul(
                            wg, lhsT=ohtg[:, j * P:(j + 1) * P], rhs=wst,
                            start=True, stop=True,
                        )
                        nc.tensor.matmul(
                            bg, lhsT=ohtg[0:64, j * P:(j + 1) * P], rhs=bst,
                            start=True, stop=True,
                        )
                        nc.vector.tensor_mul(out=xt[:, j, :], in0=xt[:, j, :], in1=wg)
                        nc.vector.tensor_add(out=xt[:, j, :], in0=xt[:, j, :], in1=bg)
                    nc.scalar.dma_start(out=out_v[g], in_=xt)

    nc.to_json_bytes = types.MethodType(_patched_to_json_bytes, nc)
    return nc


_NC = None


def _get_nc():
    global _NC
    if _NC is None:
        _NC = build_nc()
    return _NC


def _host_prep(samples, contexts, gamma, beta, priors):
    samples = np.ascontiguousarray(np.asarray(samples, dtype=np.float32))
    contexts = np.asarray(contexts, dtype=np.int32)
    gamma = np.ascontiguousarray(np.asarray(gamma, dtype=np.float32))
    beta = np.ascontiguousarray(np.asarray(beta, dtype=np.float32))
    priors = np.asarray(priors, dtype=np.float32)

    counts = np.bincount(contexts, minlength=K).astype(np.float32)[:K]
    present = (counts > 0).astype(np.float32)
    invn = (1.0 / np.maximum(counts, 1.0)).astype(np.float32).reshape(K, 1)
    psp = (present / np.sqrt(priors)).astype(np.float32).reshape(K, 1)
    ompres = (1.0 - present).astype(np.float32).reshape(K, 1)

    import ml_dtypes

    in_maps = []
    for r in range(NCORES):
        lo, hi = r * NLOC, (r + 1) * NLOC
        ctx_shard = contexts[lo:hi]
        one = (ctx_shard[:, None] == np.arange(K, dtype=np.int32)).astype(np.float32)
        oh_p = np.ascontiguousarray(
            one.reshape(NCHUNK, P, K).transpose(1, 0, 2).reshape(P, NCHUNK * K)
        )
        ohT = one.T.reshape(K, NCHUNK * P)
        ohT3_p = np.zeros((96, NCHUNK * P), dtype=ml_dtypes.bfloat16)
        for s in range(3):
            ohT3_p[32 * s:32 * s + K] = ohT.astype(ml_dtypes.bfloat16)
        ohT3_p = np.ascontiguousarray(ohT3_p)
        in_maps.append({
            "x": samples[lo:hi],
            "oh_p": oh_p,
            "ohT3_p": ohT3_p,
            "gamma": gamma,
            "beta": beta,
            "invn": invn,
            "psp": psp,
            "ompres": ompres,
        })
    return in_maps


def run(samples, contexts, gamma, beta, priors, **run_kwargs):
    """Run the SPMD kernel; returns (full_output, BassKernelResults)."""
    in_maps = _host_prep(samples, contexts, gamma, beta, priors)
    nc = _get_nc()
    res = run_bass_kernel_spmd(nc, in_maps, core_ids=list(range(NCORES)), **run_kwargs)
    out = np.concatenate([res.results[r]["out"] for r in range(NCORES)], axis=0)
    return np.ascontiguousarray(out, dtype=np.float32), res


def kernel(samples, contexts, gamma, beta, priors):
    out, _ = run(samples, contexts, gamma, beta, priors)
    return out
